# revision 1
# baseline (speedup 1.0000x reference)
"""Trainium2 Bass kernel for nn_MultiHeadAttention_32031866093611.

Sharding: pure data parallel — batch b -> NeuronCore b (B == n_cores == 8).
Weights replicated. No collectives.

Per-core program (batch b, S=1024, D=1024, H=16, DK=64), all matmuls fp32r:

  inputs (per core): xT = x[b].T [D, S], Wq/Wk/Wv/Wo [D, D] (as stored),
                     bq/bk/bv/bo [1, D], masks (host-built from prefix[b]).

  qT[c]   = (Wq[:, c*128:+128]).T @ xT + bq       -> [128 d', 1024 s]   (8 chunks)
  kT[c]   = same with Wk                          -> [128 d', 1024 s]
  v[sc]   = (xT[:, sc*128:+128]).T @ Wv + bv      -> [128 s, 16, 64+1]  (ones col)
  per head h (c=h//2, r=h%2*64):
    for kc in 0..7:
      sT[kc] = kT[c][r:r+64, kc*128:+128].T @ qT[c][r:r+64, :]   # [128 k, 1024 q]
      sT[kc] += diag/column additive masks (DVE, on cols >= kc*128)
      eT[kc] = exp(sT[kc])                                        # ACT, psum->sbuf
      outT  += v[kc][:, h, :].T @ eT[kc]       # [65, 1024]: row 64 = softmax denom
    attnT[c][r:r+64, :] = outT[0:64, :] * bcast(1/outT[64, :])
  out[sc] = (attnT[.][:, sc*128:+128]).T @ Wo + bo  -> [128 s, 1024 d] -> DRAM

The mask allowed(q,k) = (q < prefix) | (k >= q) decomposes in the transposed
[k, q] tile grid as: blocks kc > qc fully allowed (untouched); everything at or
below the diagonal (cols >= kc*128) gets one multiplicative 0/1 u8 mask applied
to the exp output on DVE (exp(s)*m == exp(s + additive mask)).

Schedule: flat (h, kc) stream with PV matmuls lagging scores/exp by 3 tiles
(in-order PE never waits on a just-issued exp); o_proj chunk k (which only
needs heads 2k, 2k+1 after the interleave) is emitted two heads after head
2k+1 retires, inside the ACT-bound attention phase; ~20 warm-up matmuls keep
the PE HAM clock-gate hot while the first x/Wq DMAs land.
"""

import numpy as np

import concourse.bass as bass
import concourse.mybir as mybir
import concourse.tile as tile
from concourse import bacc
from concourse.bass_utils import run_bass_kernel_spmd

B, S, D, H = 8, 1024, 1024, 16
DK = D // H  # 64
P = 128
NCHUNK = S // P  # 8
NCORES = 8
F32R = mybir.dt.float32r
F32 = mybir.dt.float32
EXP = mybir.ActivationFunctionType.Exp
NEG = -1.0e30
HALF = 512  # fp32 moving-operand max
MSK_OFF = [0]
for _kc in range(1, 8):
    MSK_OFF.append(MSK_OFF[-1] + S - (_kc - 1) * P)

_CACHED = {}


def build_nc(repeats=1):
    nc = bacc.Bacc("TRN2", target_bir_lowering=False, debug=False, num_devices=NCORES)

    xt_d = nc.dram_tensor("xt", [D, S], F32R, kind="ExternalInput").ap()
    wq_d = nc.dram_tensor("wq", [D, D], F32R, kind="ExternalInput").ap()
    wk_d = nc.dram_tensor("wk", [D, D], F32R, kind="ExternalInput").ap()
    wv_d = nc.dram_tensor("wv", [D, D], F32R, kind="ExternalInput").ap()
    wo_d = nc.dram_tensor("wo", [D, D], F32R, kind="ExternalInput").ap()
    bqk_d = nc.dram_tensor("bqk", [P, 2 * NCHUNK], F32, kind="ExternalInput").ap()
    ones_d = nc.dram_tensor("ones2d", [P, P], F32R, kind="ExternalInput").ap()
    bv_d = nc.dram_tensor("bv", [P, D], F32, kind="ExternalInput").ap()
    bo_d = nc.dram_tensor("bo", [P, D], F32, kind="ExternalInput").ap()
    msk_d = nc.dram_tensor("mask8", [P, 4608], mybir.dt.uint8, kind="ExternalInput").ap()
    out_d = nc.dram_tensor("out", [S, D], F32, kind="ExternalOutput").ap()

    with tile.TileContext(nc) as tc:
        with (
            tc.tile_pool(name="w", bufs=18) as wpool,
            tc.tile_pool(name="big", bufs=2) as bigpool,
            tc.tile_pool(name="qk", bufs=8) as qkpool,
            tc.tile_pool(name="v", bufs=8) as vpool,
            tc.tile_pool(name="cst", bufs=1) as cstpool,
            tc.tile_pool(name="exp", bufs=5) as exppool,
            tc.tile_pool(name="rcp", bufs=1) as rcppool,
            tc.tile_pool(name="rbc", bufs=1) as rbcpool,
            tc.tile_pool(name="osb", bufs=1) as osbpool,
            tc.tile_pool(name="pp", bufs=2, space="PSUM") as pp,
            tc.tile_pool(name="po", bufs=2, space="PSUM") as po,
        ):
            for _rep in range(repeats):
                # ---- x chunks + Wq strips interleaved (fast PE start), cst after ----
                ones2d = cstpool.tile([P, P], F32R, tag="ones2d")
                nc.sync.dma_start(ones2d[:], ones_d[:])
                ones = ones2d[0:1, :]
                xtq = [
                    bigpool.tile([P, 4, S], F32R, tag="big", name=f"xtq_{g}")
                    for g in range(2)
                ]
                def whalf(nm, w_dram, hf):
                    """8 half-strips [128, 512] of W columns [hf*512, (hf+1)*512)."""
                    ts = [
                        wpool.tile([P, HALF], F32R, tag="w", name=f"{nm}{hf}_{dc}")
                        for dc in range(NCHUNK)
                    ]
                    sl = slice(hf * HALF, (hf + 1) * HALF)
                    for dc in range(NCHUNK):
                        nc.sync.dma_start(ts[dc][:], w_dram[dc * P : (dc + 1) * P, sl])
                    return ts

                for dc in range(NCHUNK):
                    nc.sync.dma_start(
                        xtq[dc // 4][:, dc % 4, 0:HALF],
                        xt_d[dc * P : (dc + 1) * P, 0:HALF],
                    )
                qh0 = whalf("wq", wq_d, 0)
                for dc in range(NCHUNK):
                    nc.sync.dma_start(
                        xtq[dc // 4][:, dc % 4, HALF:S],
                        xt_d[dc * P : (dc + 1) * P, HALF:S],
                    )
                kh0 = whalf("wk", wk_d, 0)
                xt = [xtq[dc // 4][:, dc % 4, :] for dc in range(NCHUNK)]

                # PE warm-up: ~20 throwaway matmuls on the first-arriving tiny
                # tile keep the HAM clock-gate busy while x/Wq stream in.
                wps = pp.tile([P, S], F32, tag="pp", name="warmup_ps")
                for wi in range(18):
                    nc.tensor.matmul(
                        wps[:, 0:P], ones2d[:], ones2d[:], start=True, stop=True
                    )
                bqk = cstpool.tile([P, 2 * NCHUNK], F32, tag="bqk")
                nc.sync.dma_start(bqk[:], bqk_d[:])
                msk = cstpool.tile([P, 4608], mybir.dt.uint8, tag="msk")
                nc.sync.dma_start(msk[:], msk_d[:])
                bias = {}
                # bv (v-proj) and bo (o_proj) lifetimes don't overlap: share slot
                bias["bv"] = cstpool.tile([P, D], F32, tag="bvbo", name="bv_bc")
                nc.sync.dma_start(bias["bv"][:], bv_d[:])

                # ---- helper: dense [d', s] projection (qT / kT) ----
                def proj_half(whalf_tiles, chalf, bcol0, out_tag):
                    """qT/kT chunks chalf*4 .. chalf*4+3 from one W column half."""
                    outs = []
                    for cp in range(2):
                        cs = (chalf * 4 + 2 * cp, chalf * 4 + 2 * cp + 1)
                        pss = {
                            c: pp.tile([P, S], F32, tag="pp", name=f"ps_{out_tag}_{c}")
                            for c in cs
                        }
                        for j in range(2):
                            sl = slice(j * HALF, (j + 1) * HALF)
                            for c in cs:
                                lc = (c % 4) * P
                                for dc in range(NCHUNK):
                                    nc.tensor.matmul(
                                        pss[c][:, sl],
                                        whalf_tiles[dc][:, lc : lc + P],
                                        xt[dc][:, sl],
                                        start=(dc == 0),
                                        stop=(dc == NCHUNK - 1),
                                    )
                        for c in cs:
                            o = qkpool.tile(
                                [P, S], F32R, tag=out_tag, name=f"{out_tag}_{c}"
                            )
                            nc.vector.tensor_add(
                                o[:],
                                pss[c][:],
                                bqk[:, bcol0 + c : bcol0 + c + 1].to_broadcast((P, S)),
                            )
                            outs.append(o)
                    return outs

                with nc.named_scope("qk_proj"):
                    qT = proj_half(qh0, 0, 0, "qT")
                    qh1 = whalf("wq", wq_d, 1)
                    kT = proj_half(kh0, 0, NCHUNK, "kT")
                    kh1 = whalf("wk", wk_d, 1)
                    qT += proj_half(qh1, 1, 0, "qT")
                    kT += proj_half(kh1, 1, NCHUNK, "kT")

                # ---- v projection: [s, 16, 65] with ones column ----
                with nc.named_scope("v_proj"):
                    vh = [whalf("wv", wv_d, 0), whalf("wv", wv_d, 1)]
                    vtiles = []
                    for sc in range(NCHUNK):
                        ps = pp.tile([P, S], F32, tag="pp")
                        for j in range(2):
                            sl = slice(j * HALF, (j + 1) * HALF)
                            for dc in range(NCHUNK):
                                nc.tensor.matmul(
                                    ps[:, sl],
                                    xt[dc][:, sc * P : (sc + 1) * P],
                                    vh[j][dc][:],
                                    start=(dc == 0),
                                    stop=(dc == NCHUNK - 1),
                                )
                        vt = vpool.tile([P, H, DK + 1], F32R, tag="v")
                        nc.vector.tensor_add(
                            vt[:, :, 0:DK],
                            ps[:].rearrange("p (h d) -> p h d", h=H),
                            bias["bv"][:].rearrange("p (h d) -> p h d", h=H),
                        )
                        nc.vector.tensor_copy(
                            vt[:, :, DK : DK + 1], ones2d[:, 0:1].to_broadcast((P, H, 1))
                        )
                        vtiles.append(vt)

                # ---- attention heads ----
                bias["bo"] = cstpool.tile([P, D], F32, tag="bvbo", name="bo_bc")
                nc.sync.dma_start(bias["bo"][:], bo_d[:])
                attn = [None, None]

                # Wo strips prefetched before the head loop (slots free up as
                # Wq/Wk strips retire); o_proj chunk sc only needs heads 2sc,2sc+1.
                oh = [whalf("wo", wo_d, 0), whalf("wo", wo_d, 1)]

                def emit_scores_exp(h, kc):
                    """scores on PE, exp on ACT, multiplicative 0/1 masks on DVE."""
                    c, r = h // 2, (h % 2) * DK
                    pss = pp.tile([P, S], F32, tag="pp", name=f"pss_{h}_{kc}")
                    lhs = kT[c][r : r + DK, kc * P : (kc + 1) * P]
                    for j in range(2):
                        sl = slice(j * HALF, (j + 1) * HALF)
                        nc.tensor.matmul(
                            pss[:, sl],
                            lhs,
                            qT[c][r : r + DK, sl],
                            start=True,
                            stop=True,
                        )
                    et = exppool.tile([P, S], F32R, tag="exp", name=f"et_{h}_{kc}")
                    nc.scalar.activation(et[:], pss[:], EXP)
                    # one 0/1 mask mult over cols [kc*128, 1024): diag pattern on
                    # the diagonal block, column mask below the diagonal
                    w = S - kc * P
                    off = MSK_OFF[kc]
                    nc.vector.tensor_mul(
                        et[:, kc * P : S], et[:, kc * P : S], msk[:, off : off + w]
                    )
                    return et

                def emit_pv(h, kc, pso, et):
                    for j in range(2):
                        sl = slice(j * HALF, (j + 1) * HALF)
                        nc.tensor.matmul(
                            pso[0 : DK + 1, sl],
                            vtiles[kc][:, h, :],
                            et[:, sl],
                            start=(kc == 0),
                            stop=(kc == NCHUNK - 1),
                        )

                def emit_norm(h, pso):
                    rcp = rcppool.tile([1, S], F32, tag="rcp", name=f"rcp_{h}")
                    nc.vector.reciprocal(rcp[:], pso[DK : DK + 1, :])
                    rbc = rbcpool.tile([DK, S], F32, tag="rbc", name=f"rbc_{h}")
                    nc.gpsimd.partition_broadcast(rbc[:], rcp[:])
                    # attn[g][e*64+d, cc, h*64+u] = O_h[u*16 + 2*(4g+cc) + e, d]/denom
                    src = pso[0:DK, :].rearrange("d (u j) -> d j u", j=16)
                    rbs = rbc[:].rearrange("d (u j) -> d j u", j=16)
                    for g in range(2):
                        if attn[g] is None:
                            attn[g] = bigpool.tile(
                                [P, 4, S], F32R, tag="big", name=f"attnq_{g}"
                            )
                        for e in range(2):
                            jsl = slice(8 * g + e, 8 * (g + 1), 2)
                            nc.vector.tensor_mul(
                                attn[g][e * DK : (e + 1) * DK, :, h * DK : (h + 1) * DK],
                                src[:, jsl, :],
                                rbs[:, jsl, :],
                            )

                def emit_oproj(sc):
                    ps = po.tile([P, S], F32, tag="po", name=f"psf_{sc}")
                    for j in range(2):
                        sl = slice(j * HALF, (j + 1) * HALF)
                        for cc in range(NCHUNK):
                            nc.tensor.matmul(
                                ps[:, sl],
                                attn[cc // 4][:, cc % 4, sc * P : (sc + 1) * P],
                                oh[j][cc][:],
                                start=(cc == 0),
                                stop=(cc == NCHUNK - 1),
                            )
                    ot = osbpool.tile([P, S], F32, tag="osb", name=f"ot_{sc}")
                    nc.vector.tensor_add(ot[:], ps[:], bias["bo"][:])
                    nc.sync.dma_start(out_d[sc * P : (sc + 1) * P, :], ot[:])

                # Flat (h, kc) stream, PV lagging scores/exp by one tile so the
                # in-order PE never waits on a just-issued exp. After the last
                # PV of a head, the accumulator is copied to SBUF immediately to
                # free its PSUM bank; the norm chain reads the copy. o_proj
                # chunk k (needs heads 2k,2k+1 only) is emitted two heads later.
                from collections import deque
                pend = deque()
                pso_cur = None

                def pop_pv():
                    ph, pkc, ppso, pet = pend.popleft()
                    emit_pv(ph, pkc, ppso, pet)
                    if pkc == NCHUNK - 1:
                        emit_norm(ph, ppso)
                        if ph % 2 == 1 and ph >= 3:
                            emit_oproj((ph - 3) // 2)

                for h in range(H):
                    pso_cur = po.tile([P, S], F32, tag="po", name=f"pso_{h}")
                    for kc in range(NCHUNK):
                        et = emit_scores_exp(h, kc)
                        if len(pend) >= 4:
                            pop_pv()
                        pend.append((h, kc, pso_cur, et))
                while len(pend) > 1:
                    pop_pv()
                # last PV of head 15: slot o_proj(6) in front of the norm chain
                # so the PE stays busy while recip/bcast run on DVE/Pool.
                ph, pkc, ppso, pet = pend.popleft()
                emit_pv(ph, pkc, ppso, pet)
                emit_oproj(NCHUNK - 2)
                emit_norm(ph, ppso)
                emit_oproj(NCHUNK - 1)

    nc.compile()
    return nc


def _host_masks(prefix_b: int):
    """Combined multiplicative 0/1 mask, u8, applied to exp output.

    For scores-T tile kc (cols q in [kc*128, 1024)): element (i, q) keeps
    exp iff allowed(q, k=kc*128+i) = (q < prefix) or (k >= q).
    Segment kc occupies msk[:, off_kc : off_kc + (1024 - kc*128)].
    """
    i = np.arange(P)[:, None]
    segs = []
    for kc in range(NCHUNK):
        q = np.arange(kc * P, S)[None, :]
        k = kc * P + i
        allowed = (q < prefix_b) | (k >= q)
        segs.append(allowed.astype(np.uint8))
    return np.concatenate(segs, axis=1)


def kernel(x, prefix, Wq, bq, Wk, bk, Wv, bv, Wo, bo, _trace=False):
    x = np.asarray(x, dtype=np.float32)
    prefix = np.asarray(prefix)
    Wq, Wk, Wv, Wo = (np.ascontiguousarray(np.asarray(w, np.float32)) for w in (Wq, Wk, Wv, Wo))
    bv, bo = (
        np.broadcast_to(np.asarray(v, np.float32).reshape(1, D), (P, D)).copy()
        for v in (bv, bo)
    )
    bqk = np.stack(
        [np.asarray(bq, np.float32).reshape(NCHUNK, P), np.asarray(bk, np.float32).reshape(NCHUNK, P)], axis=0
    ).reshape(2 * NCHUNK, P).T.copy()  # [128, 16]: cols 0-7 = bq chunks, 8-15 = bk

    ones2d = np.ones((P, P), dtype=np.float32)
    if "nc" not in _CACHED:
        _CACHED["nc"] = build_nc()
    nc = _CACHED["nc"]

    in_maps = []
    for b in range(B):
        mask8 = _host_masks(int(prefix[b]))
        in_maps.append(
            {
                "xt": np.ascontiguousarray(x[b].T),
                "wq": Wq, "wk": Wk, "wv": Wv, "wo": Wo,
                "bqk": bqk, "bv": bv, "bo": bo, "ones2d": ones2d,
                "mask8": mask8,
            }
        )

    res = run_bass_kernel_spmd(nc, in_maps, core_ids=list(range(NCORES)), trace=_trace)
    out = np.stack([res.results[b]["out"] for b in range(B)], axis=0)
    if _trace:
        return out, res
    return out



# revision 4
# speedup vs baseline: 1.0186x; 1.0186x over previous
"""Trainium2 Bass kernel for nn_MultiHeadAttention_32031866093611.

Sharding: pure data parallel — batch b -> NeuronCore b (B == n_cores == 8).
Weights replicated. No collectives.

Per-core program (batch b, S=1024, D=1024, H=16, DK=64), matmuls fp32r except
the PV stage which runs bf16 (exp output + v tiles), all PSUM accum fp32:

  qT[c] = (Wq[:, c*128:+128]).T @ xT + bq  -> [128 d', 1024 s]  (ACT Identity+bias)
  kT[c] = (Wk[:, c*128:+128]).T @ xT       -> [128 d', 1024 s]  (ACT Copy; bk is
          dropped exactly: softmax over k is invariant to the q·bk term)
  v[sc] = (xT[:, sc*128:+128]).T @ Wv      -> [128 s, 16, 64+1] bf16 (ones col;
          bv is folded on host into bo_eff = bv @ Wo + bo, exact since sum(p)=1)
  per head h (c=h//2, r=h%2*64), kc DESCENDING 7..0 with width W[kc] =
  max(max_prefix, (kc+1)*128)  (cols >= W[kc] are masked on every core):
    sT[kc] = kT[c][r:r+64, kc*128:+128].T @ qT[c][r:r+64, 0:W]   # [128 k, W q]
    eT[kc] = exp(sT[kc]) -> bf16                                  # ACT
    eT[kc][:, kc*128:W] *= mask (bf16 0/1, host-built, 2x DVE mode)
    outT  += v[kc][:, h, :].T @ eT[kc][:, 0:W]   # [65, W]; row 64 = denom
  attnT[c][r:r+64, :] = outT[0:64, :] / bcast(outT[64, :])  # Pool bcast + DVE div
  out[sc] = (attnT[.][:, sc*128:+128]).T @ Wo + bo_eff -> [128 s, 1024 d] -> DRAM

The mask allowed(q,k) = (q < prefix) | (k >= q): in the transposed [k, q] grid,
cols q < kc*128 are fully allowed (untouched); cols in [kc*128, W[kc]) carry a
host-built per-core 0/1 bf16 mask; cols >= W[kc] are fully masked on every core
(prefix <= max_prefix) and are skipped in scores/exp/PV entirely — PV
accumulates kc=7 (full width, start) down to kc=0, with per-PSUM-bank stop
flags.

Schedule: flat (h, kc) stream with PV lagging scores/exp by 3 tiles; o_proj
chunk k emitted two heads after head 2k+1 retires; ~18 warm-up matmuls keep the
PE HAM clock-gate hot while the first x/Wq DMAs land.
"""

import numpy as np
import ml_dtypes

import concourse.bass as bass
import concourse.mybir as mybir
import concourse.tile as tile
from concourse import bacc
from concourse.bass_utils import run_bass_kernel_spmd

B, S, D, H = 8, 1024, 1024, 16
DK = D // H  # 64
P = 128
NCH = S // P  # 8
NCORES = 8
F32R = mybir.dt.float32r
F32 = mybir.dt.float32
BF16 = mybir.dt.bfloat16
EXP = mybir.ActivationFunctionType.Exp
IDENT = mybir.ActivationFunctionType.Identity
COPY = mybir.ActivationFunctionType.Copy
HALF = 512  # fp32 moving-operand max / one PSUM bank of fp32

_CACHED = {}


def _widths(pmax):
    """Score/exp/PV column widths per k-tile; W[7] == 1024 always."""
    return [max(pmax, (kc + 1) * P) for kc in range(NCH)]


def build_nc(pmax):
    Wd = _widths(pmax)
    mskw = [Wd[kc] - kc * P for kc in range(NCH)]
    moff = [0]
    for w in mskw[:-1]:
        moff.append(moff[-1] + w)
    msk_total = sum(mskw)
    # last-executed writer of PSUM bank 1 in the descending-kc PV group
    last_b1 = min(kc for kc in range(NCH) if Wd[kc] > HALF)

    nc = bacc.Bacc("TRN2", target_bir_lowering=False, debug=False, num_devices=NCORES)

    xt_d = nc.dram_tensor("xt", [D, S], F32R, kind="ExternalInput").ap()
    wq_d = nc.dram_tensor("wq", [D, D], F32R, kind="ExternalInput").ap()
    wk_d = nc.dram_tensor("wk", [D, D], F32R, kind="ExternalInput").ap()
    wv_d = nc.dram_tensor("wv", [D, D], F32R, kind="ExternalInput").ap()
    wo_d = nc.dram_tensor("wo", [D, D], F32R, kind="ExternalInput").ap()
    bq_d = nc.dram_tensor("bq8", [P, NCH], F32, kind="ExternalInput").ap()
    ones_d = nc.dram_tensor("ones2d", [P, P], F32R, kind="ExternalInput").ap()
    boe_d = nc.dram_tensor("boe", [P, D], F32, kind="ExternalInput").ap()
    msk_d = nc.dram_tensor("mskb", [P, msk_total], BF16, kind="ExternalInput").ap()
    out_d = nc.dram_tensor("out", [S, D], F32, kind="ExternalOutput").ap()

    with tile.TileContext(nc) as tc:
        with (
            tc.tile_pool(name="w", bufs=18) as wpool,
            tc.tile_pool(name="big", bufs=2) as bigpool,
            tc.tile_pool(name="qk", bufs=8) as qkpool,
            tc.tile_pool(name="v", bufs=8) as vpool,
            tc.tile_pool(name="cst", bufs=1) as cstpool,
            tc.tile_pool(name="exp", bufs=5) as exppool,
            tc.tile_pool(name="rcp", bufs=2) as rcppool,
            tc.tile_pool(name="rbc", bufs=2) as rbcpool,
            tc.tile_pool(name="osb", bufs=1) as osbpool,
            tc.tile_pool(name="pp", bufs=2, space="PSUM") as pp,
            tc.tile_pool(name="po", bufs=2, space="PSUM") as po,
        ):
            # ---- x chunks + Wq/Wk strips interleaved (fast PE start) ----
            ones2d = cstpool.tile([P, P], F32R, tag="ones2d")
            nc.sync.dma_start(ones2d[:], ones_d[:])
            xtq = [
                bigpool.tile([P, 4, S], F32R, tag="big", name=f"xtq_{g}")
                for g in range(2)
            ]

            def whalf(nm, w_dram, hf):
                """8 half-strips [128, 512] of W columns [hf*512, (hf+1)*512)."""
                ts = [
                    wpool.tile([P, HALF], F32R, tag="w", name=f"{nm}{hf}_{dc}")
                    for dc in range(NCH)
                ]
                sl = slice(hf * HALF, (hf + 1) * HALF)
                for dc in range(NCH):
                    nc.sync.dma_start(ts[dc][:], w_dram[dc * P : (dc + 1) * P, sl])
                return ts

            for dc in range(NCH):
                nc.sync.dma_start(
                    xtq[dc // 4][:, dc % 4, 0:HALF],
                    xt_d[dc * P : (dc + 1) * P, 0:HALF],
                )
            qh0 = whalf("wq", wq_d, 0)
            for dc in range(NCH):
                nc.sync.dma_start(
                    xtq[dc // 4][:, dc % 4, HALF:S],
                    xt_d[dc * P : (dc + 1) * P, HALF:S],
                )
            kh0 = whalf("wk", wk_d, 0)
            xt = [xtq[dc // 4][:, dc % 4, :] for dc in range(NCH)]

            # PE warm-up: throwaway matmuls on the first-arriving tiny tile
            # keep the HAM clock-gate busy while x/Wq stream in.
            wps = pp.tile([P, S], F32, tag="pp", name="warmup_ps")
            for wi in range(18):
                nc.tensor.matmul(
                    wps[:, 0:P], ones2d[:], ones2d[:], start=True, stop=True
                )
            bq8 = cstpool.tile([P, NCH], F32, tag="bq8")
            nc.sync.dma_start(bq8[:], bq_d[:])
            msk = cstpool.tile([P, msk_total], BF16, tag="msk")
            nc.sync.dma_start(msk[:], msk_d[:])

            # ---- dense [d', s] projections (qT with bias on ACT, kT copy) ----
            def proj_half(whalf_tiles, chalf, kind, out_tag):
                outs = []
                for cp in range(2):
                    cs = (chalf * 4 + 2 * cp, chalf * 4 + 2 * cp + 1)
                    pss = {
                        c: pp.tile([P, S], F32, tag="pp", name=f"ps_{out_tag}_{c}")
                        for c in cs
                    }
                    for j in range(2):
                        sl = slice(j * HALF, (j + 1) * HALF)
                        for c in cs:
                            lc = (c % 4) * P
                            for dc in range(NCH):
                                nc.tensor.matmul(
                                    pss[c][:, sl],
                                    whalf_tiles[dc][:, lc : lc + P],
                                    xt[dc][:, sl],
                                    start=(dc == 0),
                                    stop=(dc == NCH - 1),
                                )
                    for c in cs:
                        o = qkpool.tile(
                            [P, S], F32R, tag=out_tag, name=f"{out_tag}_{c}"
                        )
                        if kind == "q":
                            nc.scalar.activation(
                                o[:], pss[c][:], IDENT, bias=bq8[:, c : c + 1]
                            )
                        else:
                            nc.scalar.activation(o[:], pss[c][:], COPY)
                        outs.append(o)
                return outs

            with nc.named_scope("qk_proj"):
                qT = proj_half(qh0, 0, "q", "qT")
                qh1 = whalf("wq", wq_d, 1)
                kT = proj_half(kh0, 0, "k", "kT")
                kh1 = whalf("wk", wk_d, 1)
                qT += proj_half(qh1, 1, "q", "qT")
                kT += proj_half(kh1, 1, "k", "kT")

            # ---- v projection: [s, 16, 65] bf16 with ones column ----
            with nc.named_scope("v_proj"):
                vh = [whalf("wv", wv_d, 0), whalf("wv", wv_d, 1)]
                vtiles = []
                for sc in range(NCH):
                    ps = pp.tile([P, S], F32, tag="pp")
                    for j in range(2):
                        sl = slice(j * HALF, (j + 1) * HALF)
                        for dc in range(NCH):
                            nc.tensor.matmul(
                                ps[:, sl],
                                xt[dc][:, sc * P : (sc + 1) * P],
                                vh[j][dc][:],
                                start=(dc == 0),
                                stop=(dc == NCH - 1),
                            )
                    vt = vpool.tile([P, H, DK + 1], BF16, tag="v")
                    nc.scalar.activation(
                        vt[:, :, 0:DK], ps[:].rearrange("p (h d) -> p h d", h=H), COPY
                    )
                    nc.vector.memset(vt[:, :, DK : DK + 1], 1.0)
                    vtiles.append(vt)

            # ---- attention heads ----
            boe = cstpool.tile([P, D], F32, tag="boe")
            nc.sync.dma_start(boe[:], boe_d[:])
            attn = [None, None]

            # Wo strips prefetched before the head loop (slots free up as
            # Wq/Wk strips retire); o_proj chunk sc only needs heads 2sc,2sc+1.
            oh = [whalf("wo", wo_d, 0), whalf("wo", wo_d, 1)]

            def emit_scores_exp(h, kc):
                """scores on PE, exp->bf16 on ACT, 0/1 bf16 mask mult on DVE."""
                c, r = h // 2, (h % 2) * DK
                Wc = Wd[kc]
                pss = pp.tile([P, S], F32, tag="pp", name=f"pss_{h}_{kc}")
                lhs = kT[c][r : r + DK, kc * P : (kc + 1) * P]
                p0 = min(Wc, HALF)
                nc.tensor.matmul(
                    pss[:, 0:p0], lhs, qT[c][r : r + DK, 0:p0], start=True, stop=True
                )
                if Wc > HALF:
                    nc.tensor.matmul(
                        pss[:, HALF:Wc],
                        lhs,
                        qT[c][r : r + DK, HALF:Wc],
                        start=True,
                        stop=True,
                    )
                et = exppool.tile([P, S], BF16, tag="exp", name=f"et_{h}_{kc}")
                nc.scalar.activation(et[:, 0:Wc], pss[:, 0:Wc], EXP)
                off = moff[kc]
                w = Wc - kc * P
                nc.vector.tensor_mul(
                    et[:, kc * P : Wc], et[:, kc * P : Wc], msk[:, off : off + w]
                )
                return et

            def emit_pv(h, kc, pso, et):
                Wc = Wd[kc]
                vs = vtiles[kc][:, h, :]
                p0 = min(Wc, HALF)
                nc.tensor.matmul(
                    pso[0 : DK + 1, 0:p0],
                    vs,
                    et[:, 0:p0],
                    start=(kc == NCH - 1),
                    stop=(kc == 0),
                )
                if Wc > HALF:
                    nc.tensor.matmul(
                        pso[0 : DK + 1, HALF:Wc],
                        vs,
                        et[:, HALF:Wc],
                        start=(kc == NCH - 1),
                        stop=(kc == last_b1),
                    )

            def emit_norm(h, pso):
                """Recip on DVE (PSUM->SBUF), bcast on Pool, muls on DVE.

                attn[g][e*64+d, cc, h*64+u] = O_h[u*16 + 2*(4g+cc) + e, d]/denom
                """
                rcp = rcppool.tile([1, S], F32, tag="rcp", name=f"rcp_{h}")
                nc.vector.reciprocal(rcp[:], pso[DK : DK + 1, :])
                rbc = rbcpool.tile([DK, S], F32, tag="rbc", name=f"rbc_{h}")
                nc.gpsimd.partition_broadcast(rbc[:], rcp[:])
                src = pso[0:DK, :].rearrange("d (u j) -> d j u", j=16)
                rbs = rbc[:].rearrange("d (u j) -> d j u", j=16)
                for g in range(2):
                    if attn[g] is None:
                        attn[g] = bigpool.tile(
                            [P, 4, S], F32R, tag="big", name=f"attnq_{g}"
                        )
                    for e in range(2):
                        jsl = slice(8 * g + e, 8 * (g + 1), 2)
                        nc.vector.tensor_mul(
                            attn[g][e * DK : (e + 1) * DK, :, h * DK : (h + 1) * DK],
                            src[:, jsl, :],
                            rbs[:, jsl, :],
                        )

            def emit_oproj(sc):
                ps = po.tile([P, S], F32, tag="po", name=f"psf_{sc}")
                for j in range(2):
                    sl = slice(j * HALF, (j + 1) * HALF)
                    for cc in range(NCH):
                        nc.tensor.matmul(
                            ps[:, sl],
                            attn[cc // 4][:, cc % 4, sc * P : (sc + 1) * P],
                            oh[j][cc][:],
                            start=(cc == 0),
                            stop=(cc == NCH - 1),
                        )
                ot = osbpool.tile([P, S], F32, tag="osb", name=f"ot_{sc}")
                nc.vector.tensor_add(ot[:], ps[:], boe[:])
                nc.sync.dma_start(out_d[sc * P : (sc + 1) * P, :], ot[:])

            # Flat (h, kc-descending) stream, PV lagging scores/exp so the
            # in-order PE never waits on a just-issued exp. o_proj chunk k
            # (needs heads 2k,2k+1 only) is emitted two heads later.
            from collections import deque

            pend = deque()

            def pop_pv():
                ph, pkc, ppso, pet = pend.popleft()
                emit_pv(ph, pkc, ppso, pet)
                if pkc == 0:
                    emit_norm(ph, ppso)
                    if ph % 2 == 1 and ph >= 3:
                        emit_oproj((ph - 3) // 2)

            for h in range(H):
                pso_cur = po.tile([P, S], F32, tag="po", name=f"pso_{h}")
                for kc in range(NCH - 1, -1, -1):
                    et = emit_scores_exp(h, kc)
                    if len(pend) >= 4:
                        pop_pv()
                    pend.append((h, kc, pso_cur, et))
            while len(pend) > 1:
                pop_pv()
            # last PV of head 15: slot o_proj(6) in front of the norm chain
            # so the PE stays busy while bcast/divide run on Pool/DVE.
            ph, pkc, ppso, pet = pend.popleft()
            emit_pv(ph, pkc, ppso, pet)
            emit_oproj(NCH - 2)
            emit_norm(ph, ppso)
            emit_oproj(NCH - 1)

    nc.compile()
    return nc


def _host_mask(prefix_b, pmax):
    """Per-core multiplicative 0/1 bf16 mask over cols [kc*128, W[kc])."""
    Wd = _widths(pmax)
    i = np.arange(P)[:, None]
    segs = []
    for kc in range(NCH):
        q = np.arange(kc * P, Wd[kc])[None, :]
        k = kc * P + i
        allowed = (q < prefix_b) | (k >= q)
        segs.append(allowed.astype(ml_dtypes.bfloat16))
    return np.concatenate(segs, axis=1)


def kernel(x, prefix, Wq, bq, Wk, bk, Wv, bv, Wo, bo, _trace=False):
    x = np.asarray(x, dtype=np.float32)
    prefix = np.asarray(prefix)
    Wq, Wk, Wv, Wo = (
        np.ascontiguousarray(np.asarray(w, np.float32)) for w in (Wq, Wk, Wv, Wo)
    )
    pmax = int(prefix.max())
    # Exact folds: softmax_k[(q+bq)·(k+bk)] == softmax_k[(q+bq)·k]  (q·bk and
    # bq·bk are constant over k); out = attn@Wo + (bv@Wo + bo) since sum(p)=1.
    boe = (
        np.asarray(bv, np.float64) @ np.asarray(Wo, np.float64) + np.asarray(bo)
    ).astype(np.float32)
    boe_bc = np.broadcast_to(boe.reshape(1, D), (P, D)).copy()
    bq8 = np.asarray(bq, np.float32).reshape(NCH, P).T.copy()  # [128, 8] cols
    ones2d = np.ones((P, P), dtype=np.float32)

    if pmax not in _CACHED:
        _CACHED[pmax] = build_nc(pmax)
    nc = _CACHED[pmax]

    in_maps = []
    for b in range(B):
        in_maps.append(
            {
                "xt": np.ascontiguousarray(x[b].T),
                "wq": Wq, "wk": Wk, "wv": Wv, "wo": Wo,
                "bq8": bq8, "boe": boe_bc, "ones2d": ones2d,
                "mskb": _host_mask(int(prefix[b]), pmax),
            }
        )

    res = run_bass_kernel_spmd(nc, in_maps, core_ids=list(range(NCORES)), trace=_trace)
    out = np.stack([res.results[b]["out"] for b in range(B)], axis=0)
    if _trace:
        return out, res
    return out


# revision 5
# speedup vs baseline: 1.0524x; 1.0332x over previous
"""Trainium2 Bass kernel for nn_MultiHeadAttention_32031866093611.

Sharding: pure data parallel — batch b -> NeuronCore b (B == n_cores == 8).
Weights replicated. No collectives.

Per-core program (batch b, S=1024, D=1024, H=16, DK=64), matmuls fp32r except
the PV stage which runs bf16 (exp output + v tiles), all PSUM accum fp32:

  qT[c] = (Wq[:, c*128:+128]).T @ xT + bq  -> [128 d', 1024 s]  (ACT Identity+bias)
  kT[c] = (Wk[:, c*128:+128]).T @ xT       -> [128 d', 1024 s]  (ACT Copy; bk is
          dropped exactly: softmax over k is invariant to the q·bk term)
  v[sc] = (xT[:, sc*128:+128]).T @ Wv      -> [128 s, 16, 64+1] bf16 (ones col;
          bv is folded on host into bo_eff = bv @ Wo + bo, exact since sum(p)=1)
  per head h (c=h//2, r=h%2*64), kc DESCENDING 7..0 with width W[kc] =
  max(max_prefix, (kc+1)*128)  (cols >= W[kc] are masked on every core):
    sT[kc] = kT[c][r:r+64, kc*128:+128].T @ qT[c][r:r+64, 0:W]   # [128 k, W q]
    eT[kc] = exp(sT[kc]) -> bf16                                  # ACT
    eT[kc][:, kc*128:W] *= mask (bf16 0/1, host-built, 4x DVE mode)
    outT  += v[kc][:, h, :].T @ eT[kc][:, 0:W]   # [65, W]; row 64 = denom
  attnT[c][r:r+64, :] = outT[0:64, :] * bcast(1/outT[64, :])
  out[sc] = (attnT[.][:, sc*128:+128]).T @ Wo + bo_eff -> [128 s, 1024 d] -> DRAM

Schedule (single in-order PE stream, PE is the binding engine at ~92% of the
kernel): per-chunk W loads let the first q/k projection start ~13us in; the v
projection tiles (descending sc, matching the descending-kc PV accumulation)
are woven between head 0's score tiles; q/k projections for chunk c+1 are
woven into head 2c+1's stream; o_proj chunk k fires two heads after head 2k+1
retires. Scores/exp/mask/PV share two PSUM score slots with the woven
projection psums (the PE never holds more than two `pp` tiles at once); PV
lags scores by 4 tiles so the in-order PE never waits on a just-issued exp.
"""

import numpy as np
import ml_dtypes

import concourse.bass as bass
import concourse.mybir as mybir
import concourse.tile as tile
from concourse import bacc
from concourse.bass_utils import run_bass_kernel_spmd

B, S, D, H = 8, 1024, 1024, 16
DK = D // H  # 64
P = 128
NCH = S // P  # 8
NCORES = 8
F32R = mybir.dt.float32r
F32 = mybir.dt.float32
BF16 = mybir.dt.bfloat16
EXP = mybir.ActivationFunctionType.Exp
IDENT = mybir.ActivationFunctionType.Identity
COPY = mybir.ActivationFunctionType.Copy
HALF = 512  # fp32 moving-operand max / one PSUM bank of fp32

_CACHED = {}


def _widths(pmax):
    """Score/exp/PV column widths per k-tile; W[7] == 1024 always."""
    return [max(pmax, (kc + 1) * P) for kc in range(NCH)]


def build_nc(pmax):
    Wd = _widths(pmax)
    mskw = [Wd[kc] - kc * P for kc in range(NCH)]
    moff = [0]
    for w in mskw[:-1]:
        moff.append(moff[-1] + w)
    msk_total = sum(mskw)
    # last-executed writer of PSUM bank 1 in the descending-kc PV group
    last_b1 = min(kc for kc in range(NCH) if Wd[kc] > HALF)

    nc = bacc.Bacc("TRN2", target_bir_lowering=False, debug=False, num_devices=NCORES)

    xt_d = nc.dram_tensor("xt", [D, S], F32R, kind="ExternalInput").ap()
    wq_d = nc.dram_tensor("wq", [D, D], F32R, kind="ExternalInput").ap()
    wk_d = nc.dram_tensor("wk", [D, D], F32R, kind="ExternalInput").ap()
    wv_d = nc.dram_tensor("wv", [D, D], F32R, kind="ExternalInput").ap()
    wo_d = nc.dram_tensor("wo", [D, D], F32R, kind="ExternalInput").ap()
    bq_d = nc.dram_tensor("bq8", [P, NCH], F32, kind="ExternalInput").ap()
    ones_d = nc.dram_tensor("ones2d", [P, P], F32R, kind="ExternalInput").ap()
    boe_d = nc.dram_tensor("boe", [P, D], F32, kind="ExternalInput").ap()
    msk_d = nc.dram_tensor("mskb", [P, msk_total], BF16, kind="ExternalInput").ap()
    out_d = nc.dram_tensor("out", [S, D], F32, kind="ExternalOutput").ap()

    with tile.TileContext(nc) as tc:
        with (
            tc.tile_pool(name="w", bufs=18) as wpool,        # wv + wo [P,512] strips
            tc.tile_pool(name="wqk", bufs=32) as wqkpool,    # per-chunk [P,P] q/k blocks
            tc.tile_pool(name="big", bufs=2) as bigpool,     # xT
            tc.tile_pool(name="atn", bufs=2) as atnpool,     # attn (own pool: xT is
            tc.tile_pool(name="qk", bufs=3) as qkpool,       #  still live at norm(0))
            tc.tile_pool(name="v", bufs=8) as vpool,
            tc.tile_pool(name="cst", bufs=1) as cstpool,
            tc.tile_pool(name="exp", bufs=5) as exppool,
            tc.tile_pool(name="rcp", bufs=2) as rcppool,
            tc.tile_pool(name="rbc", bufs=2) as rbcpool,
            tc.tile_pool(name="osb", bufs=2) as osbpool,
            tc.tile_pool(name="pp", bufs=2, space="PSUM") as pp,
            tc.tile_pool(name="po", bufs=2, space="PSUM") as po,
        ):
            # ---- DMA front: x + chunk-0 W blocks first for a fast PE start ----
            ones2d = cstpool.tile([P, P], F32R, tag="ones2d")
            nc.sync.dma_start(ones2d[:], ones_d[:])
            bq8 = cstpool.tile([P, NCH], F32, tag="bq8")
            nc.sync.dma_start(bq8[:], bq_d[:])
            xtq = [
                bigpool.tile([P, 4, S], F32R, tag="big", name=f"xtq_{g}")
                for g in range(2)
            ]

            def load_wchunk(nm, w_dram, c):
                """8 [128,128] blocks of W columns [c*128, (c+1)*128)."""
                ts = [
                    wqkpool.tile([P, P], F32R, tag="wqk", name=f"{nm}{c}_{dc}")
                    for dc in range(NCH)
                ]
                for dc in range(NCH):
                    nc.sync.dma_start(
                        ts[dc][:], w_dram[dc * P : (dc + 1) * P, c * P : (c + 1) * P]
                    )
                return ts

            def whalf(nm, w_dram, hf):
                """8 half-strips [128, 512] of W columns [hf*512, (hf+1)*512)."""
                ts = [
                    wpool.tile([P, HALF], F32R, tag="w", name=f"{nm}{hf}_{dc}")
                    for dc in range(NCH)
                ]
                sl = slice(hf * HALF, (hf + 1) * HALF)
                for dc in range(NCH):
                    nc.sync.dma_start(ts[dc][:], w_dram[dc * P : (dc + 1) * P, sl])
                return ts

            for dc in range(NCH):
                nc.sync.dma_start(
                    xtq[dc // 4][:, dc % 4, 0:HALF],
                    xt_d[dc * P : (dc + 1) * P, 0:HALF],
                )
            wqc = {0: load_wchunk("wq", wq_d, 0)}
            for dc in range(NCH):
                nc.sync.dma_start(
                    xtq[dc // 4][:, dc % 4, HALF:S],
                    xt_d[dc * P : (dc + 1) * P, HALF:S],
                )
            wkc = {0: load_wchunk("wk", wk_d, 0)}
            xt = [xtq[dc // 4][:, dc % 4, :] for dc in range(NCH)]
            msk = cstpool.tile([P, msk_total], BF16, tag="msk")
            nc.sync.dma_start(msk[:], msk_d[:])
            vh = [whalf("wv", wv_d, 0), whalf("wv", wv_d, 1)]

            # PE warm-up: throwaway matmuls on the first-arriving tiny tile
            # keep the HAM clock-gate hot while x/Wq stream in.
            wps = pp.tile([P, S], F32, tag="pp", name="warmup_ps")
            for wi in range(10):
                nc.tensor.matmul(
                    wps[:, 0:P], ones2d[:], ones2d[:], start=True, stop=True
                )

            qT, kT = {}, {}

            def proj_qk(c, kind):
                wts = (wqc if kind == "q" else wkc)[c]
                pss = pp.tile([P, S], F32, tag="pp", name=f"ps_{kind}{c}")
                for j in range(2):
                    sl = slice(j * HALF, (j + 1) * HALF)
                    for dc in range(NCH):
                        nc.tensor.matmul(
                            pss[:, sl],
                            wts[dc][:],
                            xt[dc][:, sl],
                            start=(dc == 0),
                            stop=(dc == NCH - 1),
                        )
                o = qkpool.tile(
                    [P, S], F32R, tag="qT" if kind == "q" else "kT",
                    name=f"{kind}T_{c}",
                )
                if kind == "q":
                    nc.scalar.activation(o[:], pss[:], IDENT, bias=bq8[:, c : c + 1])
                else:
                    nc.scalar.activation(o[:], pss[:], COPY)
                (qT if kind == "q" else kT)[c] = o

            vtiles = {}

            def proj_v(sc):
                ps = pp.tile([P, S], F32, tag="pp", name=f"ps_v{sc}")
                for j in range(2):
                    sl = slice(j * HALF, (j + 1) * HALF)
                    for dc in range(NCH):
                        nc.tensor.matmul(
                            ps[:, sl],
                            xt[dc][:, sc * P : (sc + 1) * P],
                            vh[j][dc][:],
                            start=(dc == 0),
                            stop=(dc == NCH - 1),
                        )
                vt = vpool.tile([P, H, DK + 1], BF16, tag="v", name=f"vt_{sc}")
                nc.scalar.activation(
                    vt[:, :, 0:DK], ps[:].rearrange("p (h d) -> p h d", h=H), COPY
                )
                nc.vector.memset(vt[:, :, DK : DK + 1], 1.0)
                vtiles[sc] = vt

            attn = [None, None]
            oh = [None, None]
            boe = cstpool.tile([P, D], F32, tag="boe")

            def emit_scores_exp(h, kc):
                """scores on PE, exp->bf16 on ACT, 0/1 bf16 mask mult on DVE."""
                c, r = h // 2, (h % 2) * DK
                Wc = Wd[kc]
                pss = pp.tile([P, S], F32, tag="pp", name=f"pss_{h}_{kc}")
                lhs = kT[c][r : r + DK, kc * P : (kc + 1) * P]
                p0 = min(Wc, HALF)
                nc.tensor.matmul(
                    pss[:, 0:p0], lhs, qT[c][r : r + DK, 0:p0], start=True, stop=True
                )
                if Wc > HALF:
                    nc.tensor.matmul(
                        pss[:, HALF:Wc],
                        lhs,
                        qT[c][r : r + DK, HALF:Wc],
                        start=True,
                        stop=True,
                    )
                et = exppool.tile([P, S], BF16, tag="exp", name=f"et_{h}_{kc}")
                nc.scalar.activation(et[:, 0:Wc], pss[:, 0:Wc], EXP)
                off = moff[kc]
                w = Wc - kc * P
                nc.vector.tensor_mul(
                    et[:, kc * P : Wc], et[:, kc * P : Wc], msk[:, off : off + w]
                )
                return et

            def emit_pv(h, kc, pso, et):
                Wc = Wd[kc]
                vs = vtiles[kc][:, h, :]
                p0 = min(Wc, HALF)
                nc.tensor.matmul(
                    pso[0 : DK + 1, 0:p0],
                    vs,
                    et[:, 0:p0],
                    start=(kc == NCH - 1),
                    stop=(kc == 0),
                )
                if Wc > HALF:
                    nc.tensor.matmul(
                        pso[0 : DK + 1, HALF:Wc],
                        vs,
                        et[:, HALF:Wc],
                        start=(kc == NCH - 1),
                        stop=(kc == last_b1),
                    )

            def emit_norm(h, pso):
                """Recip on DVE (PSUM->SBUF), bcast on Pool, muls on DVE.

                attn[g][e*64+d, cc, h*64+u] = O_h[u*16 + 2*(4g+cc) + e, d]/denom
                """
                rcp = rcppool.tile([1, S], F32, tag="rcp", name=f"rcp_{h}")
                nc.vector.reciprocal(rcp[:], pso[DK : DK + 1, :])
                rbc = rbcpool.tile([DK, S], F32, tag="rbc", name=f"rbc_{h}")
                nc.gpsimd.partition_broadcast(rbc[:], rcp[:])
                src = pso[0:DK, :].rearrange("d (u j) -> d j u", j=16)
                rbs = rbc[:].rearrange("d (u j) -> d j u", j=16)
                for g in range(2):
                    if attn[g] is None:
                        attn[g] = atnpool.tile(
                            [P, 4, S], F32R, tag="atn", name=f"attnq_{g}"
                        )
                    for e in range(2):
                        jsl = slice(8 * g + e, 8 * (g + 1), 2)
                        nc.vector.tensor_mul(
                            attn[g][e * DK : (e + 1) * DK, :, h * DK : (h + 1) * DK],
                            src[:, jsl, :],
                            rbs[:, jsl, :],
                        )

            def emit_oproj(sc):
                ps = po.tile([P, S], F32, tag="po", name=f"psf_{sc}")
                for j in range(2):
                    sl = slice(j * HALF, (j + 1) * HALF)
                    for cc in range(NCH):
                        nc.tensor.matmul(
                            ps[:, sl],
                            attn[cc // 4][:, cc % 4, sc * P : (sc + 1) * P],
                            oh[j][cc][:],
                            start=(cc == 0),
                            stop=(cc == NCH - 1),
                        )
                ot = osbpool.tile([P, S], F32, tag="osb", name=f"ot_{sc}")
                nc.vector.tensor_add(ot[:], ps[:], boe[:])
                nc.sync.dma_start(out_d[sc * P : (sc + 1) * P, :], ot[:])

            # ---- the single interleaved stream ----
            from collections import deque

            pend = deque()

            def pop_pv():
                ph, pkc, ppso, pet = pend.popleft()
                emit_pv(ph, pkc, ppso, pet)
                if pkc == 0:
                    emit_norm(ph, ppso)
                    if ph % 2 == 1 and ph >= 3:
                        emit_oproj((ph - 3) // 2)

            proj_qk(0, "q")
            proj_qk(0, "k")

            for h in range(H):
                c = h // 2
                pso_cur = po.tile([P, S], F32, tag="po", name=f"pso_{h}")
                if h == 1:
                    # W chunk blocks for the NEXT pair stream in one pair ahead
                    wqc[1] = load_wchunk("wq", wq_d, 1)
                    wkc[1] = load_wchunk("wk", wk_d, 1)
                if h >= 3 and h % 2 == 1 and c + 1 < NCH:
                    wqc[c + 1] = load_wchunk("wq", wq_d, c + 1)
                    wkc[c + 1] = load_wchunk("wk", wk_d, c + 1)
                for kc in range(NCH - 1, -1, -1):
                    et = emit_scores_exp(h, kc)
                    if len(pend) >= 4:
                        pop_pv()
                    pend.append((h, kc, pso_cur, et))
                    if h == 0:
                        # v tiles woven into head 0, descending to match PV order
                        proj_v(kc)
                        if kc == 0:
                            # Wo strips + boe: DMA queue position after wv's
                            # last use frees wpool slots
                            oh[0] = whalf("wo", wo_d, 0)
                            oh[1] = whalf("wo", wo_d, 1)
                            nc.sync.dma_start(boe[:], boe_d[:])
                    elif h % 2 == 1 and c + 1 < NCH:
                        if kc == NCH - 1:
                            proj_qk(c + 1, "q")
                        elif kc == 4:
                            proj_qk(c + 1, "k")
            while len(pend) > 1:
                pop_pv()
            # last PV of head 15: slot o_proj(6) in front of the norm chain
            # so the PE stays busy while recip/bcast run on DVE/Pool.
            ph, pkc, ppso, pet = pend.popleft()
            emit_pv(ph, pkc, ppso, pet)
            emit_oproj(NCH - 2)
            emit_norm(ph, ppso)
            emit_oproj(NCH - 1)

    nc.compile()
    return nc


def _host_mask(prefix_b, pmax):
    """Per-core multiplicative 0/1 bf16 mask over cols [kc*128, W[kc])."""
    Wd = _widths(pmax)
    i = np.arange(P)[:, None]
    segs = []
    for kc in range(NCH):
        q = np.arange(kc * P, Wd[kc])[None, :]
        k = kc * P + i
        allowed = (q < prefix_b) | (k >= q)
        segs.append(allowed.astype(ml_dtypes.bfloat16))
    return np.concatenate(segs, axis=1)


def kernel(x, prefix, Wq, bq, Wk, bk, Wv, bv, Wo, bo, _trace=False):
    x = np.asarray(x, dtype=np.float32)
    prefix = np.asarray(prefix)
    Wq, Wk, Wv, Wo = (
        np.ascontiguousarray(np.asarray(w, np.float32)) for w in (Wq, Wk, Wv, Wo)
    )
    pmax = int(prefix.max())
    # Exact folds: softmax_k[(q+bq)·(k+bk)] == softmax_k[(q+bq)·k]  (q·bk and
    # bq·bk are constant over k); out = attn@Wo + (bv@Wo + bo) since sum(p)=1.
    boe = (
        np.asarray(bv, np.float64) @ np.asarray(Wo, np.float64) + np.asarray(bo)
    ).astype(np.float32)
    boe_bc = np.broadcast_to(boe.reshape(1, D), (P, D)).copy()
    bq8 = np.asarray(bq, np.float32).reshape(NCH, P).T.copy()  # [128, 8] cols
    ones2d = np.ones((P, P), dtype=np.float32)

    if pmax not in _CACHED:
        _CACHED[pmax] = build_nc(pmax)
    nc = _CACHED[pmax]

    in_maps = []
    for b in range(B):
        in_maps.append(
            {
                "xt": np.ascontiguousarray(x[b].T),
                "wq": Wq, "wk": Wk, "wv": Wv, "wo": Wo,
                "bq8": bq8, "boe": boe_bc, "ones2d": ones2d,
                "mskb": _host_mask(int(prefix[b]), pmax),
            }
        )

    res = run_bass_kernel_spmd(nc, in_maps, core_ids=list(range(NCORES)), trace=_trace)
    out = np.stack([res.results[b]["out"] for b in range(B)], axis=0)
    if _trace:
        return out, res
    return out
